# revision 5
# baseline (speedup 1.0000x reference)
"""Bass/Trainium2 kernel for nn_BlockSSM (linear block state-space model).

Math (from the reference):
    s_{k+1} = 2*(s_k @ Wx.T + bx + u_k @ Wu.T + bu) + (d_k @ Wd.T + bd)
            = s_k @ A + u_k @ Bu + d_k @ Bd + c
      where A = 2*Wx.T, Bu = 2*Wu.T, Bd = Wd.T, c = 2*bx + 2*bu + bd
    X[k] = s_{k+1}
    Y[k] = s_{k+1} @ Wy.T + by
    reg_error: every stat is multiplied by a 0.0 coefficient (Q_sub's term is
    0.2 * sum([]) == 0), so reg_error == 0.0 exactly.

Strategy: data-parallel over batch (1024 -> 128 per core on 8 cores).
All matmuls run in bf16 with fp32 PSUM accumulation. The state transpose
needed for the next step's matmul (lhsT layout) is done with the DMA xbar
transpose engine, off the compute engines' critical path. The per-step
bias is injected with an all-ones [128,128] lhsT against a bias/128
replicated rhs, so the full state accumulates in PSUM and is evacuated
once (fp32 for the X output via ScalarE, bf16 for the recurrence via
VectorE).
"""

import sys

for _p in ("/opt/trn_rl_repo",):
    if _p not in sys.path:
        sys.path.insert(0, _p)

import numpy as np
import ml_dtypes

import concourse.bacc as bacc
import concourse.mybir as mybir
from concourse.tile import TileContext
from concourse.bass_utils import run_bass_kernel_spmd

BF16 = mybir.dt.bfloat16
F32 = mybir.dt.float32
NPBF16 = ml_dtypes.bfloat16

N_CORES = 8
T, B = 64, 1024
NX, NU, ND, NY = 1024, 512, 256, 256
BL = B // N_CORES  # 128 batch rows per core
KX, KU, KD = NX // 128, NU // 128, ND // 128  # k-tile counts: 8, 4, 2


def build(t_steps: int = T):
    nc = bacc.Bacc("TRN2", target_bir_lowering=False, debug=False,
                   num_devices=N_CORES)

    xT0 = nc.dram_tensor("xT0", [128, NX], BF16, kind="ExternalInput")
    UT = nc.dram_tensor("UT", [t_steps, 128, NU], BF16, kind="ExternalInput")
    DT = nc.dram_tensor("DT", [t_steps, 128, ND], BF16, kind="ExternalInput")
    WA = nc.dram_tensor("WA", [128, KX * NX], BF16, kind="ExternalInput")
    WU = nc.dram_tensor("WU", [128, KU * NX], BF16, kind="ExternalInput")
    WD = nc.dram_tensor("WD", [128, KD * NX], BF16, kind="ExternalInput")
    WY = nc.dram_tensor("WY", [128, KX * NY], BF16, kind="ExternalInput")
    CR = nc.dram_tensor("CR", [128, NX], BF16, kind="ExternalInput")
    BYR = nc.dram_tensor("BYR", [128, NY], BF16, kind="ExternalInput")
    ONE = nc.dram_tensor("ONE", [128, 128], BF16, kind="ExternalInput")
    XO = nc.dram_tensor("XO", [t_steps, 128, NX], F32, kind="ExternalOutput")
    YO = nc.dram_tensor("YO", [t_steps, 128, NY], F32, kind="ExternalOutput")

    with TileContext(nc) as tc:
        with (
            tc.tile_pool(name="wpool", bufs=1) as wpool,
            tc.tile_pool(name="upool", bufs=4) as upool,
            tc.tile_pool(name="dpool", bufs=4) as dpool,
            tc.tile_pool(name="xtpool", bufs=3) as xtpool,
            tc.tile_pool(name="xbfpool", bufs=3) as xbfpool,
            tc.tile_pool(name="xspool", bufs=4) as xspool,
            tc.tile_pool(name="yspool", bufs=4) as yspool,
            tc.tile_pool(name="pxpool", bufs=2, space="PSUM") as pxpool,
            tc.tile_pool(name="pypool", bufs=2, space="PSUM") as pypool,
        ):
            wa = wpool.tile([128, KX * NX], BF16, name="wa")
            nc.gpsimd.dma_start(wa[:], WA[:])
            wu = wpool.tile([128, KU * NX], BF16, name="wu")
            nc.gpsimd.dma_start(wu[:], WU[:])
            wd = wpool.tile([128, KD * NX], BF16, name="wd")
            nc.gpsimd.dma_start(wd[:], WD[:])
            wy = wpool.tile([128, KX * NY], BF16, name="wy")
            nc.gpsimd.dma_start(wy[:], WY[:])
            cr = wpool.tile([128, NX], BF16, name="cr")
            nc.gpsimd.dma_start(cr[:], CR[:])
            byr = wpool.tile([128, NY], BF16, name="byr")
            nc.gpsimd.dma_start(byr[:], BYR[:])
            one = wpool.tile([128, 128], BF16, name="one")
            nc.gpsimd.dma_start(one[:], ONE[:])

            xT_cur = xtpool.tile([128, NX], BF16, name="xT")
            nc.gpsimd.dma_start(xT_cur[:], xT0[:])

            for k in range(t_steps):
                ut = upool.tile([128, NU], BF16, name="ut")
                nc.gpsimd.dma_start(ut[:], UT[k])
                dt = dpool.tile([128, ND], BF16, name="dt")
                nc.gpsimd.dma_start(dt[:], DT[k])

                px_lo = pxpool.tile([128, 512], F32, name="px_lo")
                px_hi = pxpool.tile([128, 512], F32, name="px_hi")
                py = pypool.tile([128, NY], F32, name="py") if k >= 1 else None

                # bias rows: ones.T @ (c/128 replicated) adds c to every row
                nc.tensor.matmul(px_lo[:], lhsT=one[:], rhs=cr[:, 0:512],
                                 start=True, stop=False)
                nc.tensor.matmul(px_hi[:], lhsT=one[:], rhs=cr[:, 512:1024],
                                 start=True, stop=False)
                if py is not None:
                    nc.tensor.matmul(py[:], lhsT=one[:], rhs=byr[:],
                                     start=True, stop=False)

                # input projections (independent of the recurrence)
                for kt in range(KU):
                    lh = ut[:, kt * 128:(kt + 1) * 128]
                    nc.tensor.matmul(px_lo[:], lhsT=lh,
                                     rhs=wu[:, kt * NX:kt * NX + 512],
                                     start=False, stop=False)
                    nc.tensor.matmul(px_hi[:], lhsT=lh,
                                     rhs=wu[:, kt * NX + 512:(kt + 1) * NX],
                                     start=False, stop=False)
                for kt in range(KD):
                    lh = dt[:, kt * 128:(kt + 1) * 128]
                    nc.tensor.matmul(px_lo[:], lhsT=lh,
                                     rhs=wd[:, kt * NX:kt * NX + 512],
                                     start=False, stop=False)
                    nc.tensor.matmul(px_hi[:], lhsT=lh,
                                     rhs=wd[:, kt * NX + 512:(kt + 1) * NX],
                                     start=False, stop=False)

                # recurrence: s_{k+1} += s_k @ A   (critical path)
                for i in range(KX):
                    last = i == KX - 1
                    lh = xT_cur[:, i * 128:(i + 1) * 128]
                    nc.tensor.matmul(px_lo[:], lhsT=lh,
                                     rhs=wa[:, i * NX:i * NX + 512],
                                     start=False, stop=last)
                    nc.tensor.matmul(px_hi[:], lhsT=lh,
                                     rhs=wa[:, i * NX + 512:(i + 1) * NX],
                                     start=False, stop=last)

                # previous step's output head: Y[k-1] = s_k @ Wy.T + by
                # (uses the same lhsT tiles; off the critical path, fills
                # the PE while s_{k+1} is evacuated/transposed)
                if py is not None:
                    for i in range(KX):
                        last = i == KX - 1
                        lh = xT_cur[:, i * 128:(i + 1) * 128]
                        nc.tensor.matmul(py[:], lhsT=lh,
                                         rhs=wy[:, i * NY:(i + 1) * NY],
                                         start=False, stop=last)

                # evacuate state: bf16 for the recurrence, fp32 for X output
                xbf = xbfpool.tile([128, NX], BF16, name="xbf")
                nc.vector.tensor_copy(out=xbf[:, 0:512], in_=px_lo[:])
                nc.vector.tensor_copy(out=xbf[:, 512:1024], in_=px_hi[:])
                xstage = xspool.tile([128, NX], F32, name="xstage")
                nc.scalar.copy(out=xstage[:, 0:512], in_=px_lo[:])
                nc.scalar.copy(out=xstage[:, 512:1024], in_=px_hi[:])
                nc.gpsimd.dma_start(XO[k], xstage[:])

                if py is not None:
                    ystage = yspool.tile([128, NY], F32, name="ystage")
                    nc.scalar.copy(out=ystage[:], in_=py[:])
                    nc.gpsimd.dma_start(YO[k - 1], ystage[:])

                # transpose the new state for the next step's lhsT
                xT_next = xtpool.tile([128, NX], BF16, name="xT")
                for i in range(KX):
                    nc.sync.dma_start_transpose(
                        xT_next[:, i * 128:(i + 1) * 128],
                        xbf[:, i * 128:(i + 1) * 128])
                xT_cur = xT_next

            # epilogue: Y[T-1] = s_T @ Wy.T + by
            py = pypool.tile([128, NY], F32, name="py")
            nc.tensor.matmul(py[:], lhsT=one[:], rhs=byr[:],
                             start=True, stop=False)
            for i in range(KX):
                last = i == KX - 1
                lh = xT_cur[:, i * 128:(i + 1) * 128]
                nc.tensor.matmul(py[:], lhsT=lh,
                                 rhs=wy[:, i * NY:(i + 1) * NY],
                                 start=False, stop=last)
            ystage = yspool.tile([128, NY], F32, name="ystage")
            nc.scalar.copy(out=ystage[:], in_=py[:])
            nc.gpsimd.dma_start(YO[t_steps - 1], ystage[:])

    nc.compile()
    return nc


def _to_lhsT_tiles(mat: np.ndarray) -> np.ndarray:
    """[rows, cols] -> [128, (rows//128)*cols] laid out so that
    out[p, i*cols + j] = mat[i*128 + p, j] (k-tile-major free dim)."""
    rows, cols = mat.shape
    kt = rows // 128
    return (mat.reshape(kt, 128, cols).transpose(1, 0, 2)
            .reshape(128, kt * cols))


def _prep_weights(Wx, bx, Wu, bu, Wd, bd, Wy, by):
    A = (2.0 * Wx.T).astype(np.float32)          # [NX, NX]
    Bu = (2.0 * Wu.T).astype(np.float32)         # [NU, NX]
    Bd = Wd.T.astype(np.float32)                 # [ND, NX]
    Cy = Wy.T.astype(np.float32)                 # [NX, NY]
    c = (2.0 * bx + 2.0 * bu + bd).astype(np.float32)

    wa = _to_lhsT_tiles(A).astype(NPBF16)
    wu = _to_lhsT_tiles(Bu).astype(NPBF16)
    wd = _to_lhsT_tiles(Bd).astype(NPBF16)
    wy = _to_lhsT_tiles(Cy).astype(NPBF16)
    cr = np.broadcast_to((c / 128.0).astype(NPBF16), (128, NX)).copy()
    byr = np.broadcast_to((by.astype(np.float32) / 128.0).astype(NPBF16),
                          (128, NY)).copy()
    one = np.ones((128, 128), dtype=NPBF16)
    return dict(WA=wa, WU=wu, WD=wd, WY=wy, CR=cr, BYR=byr, ONE=one)


def _prep_seq(M: np.ndarray) -> np.ndarray:
    """[t, BL, F] fp32 -> [t, 128, F] bf16 transposed per step:
    out[t, p, kt*128+m] = M[t, m, kt*128+p]."""
    t, bl, f = M.shape
    kt = f // 128
    out = (M.transpose(0, 2, 1)                 # [t, F, BL]
           .reshape(t, kt, 128, bl)             # [t, kt, p, m]
           .transpose(0, 2, 1, 3)               # [t, p, kt, m]
           .reshape(t, 128, kt * bl))
    return np.ascontiguousarray(out).astype(NPBF16)


_NC_CACHE = {}


def _get_nc(t_steps: int = T):
    if t_steps not in _NC_CACHE:
        _NC_CACHE[t_steps] = build(t_steps)
    return _NC_CACHE[t_steps]


def kernel(x, U, D, Wx, bx, Wu, bu, Wd, bd, Wy, by, **run_kwargs):
    x = np.asarray(x, dtype=np.float32)
    U = np.asarray(U, dtype=np.float32)
    D = np.asarray(D, dtype=np.float32)
    Wx = np.asarray(Wx, dtype=np.float32)
    bx = np.asarray(bx, dtype=np.float32)
    Wu = np.asarray(Wu, dtype=np.float32)
    bu = np.asarray(bu, dtype=np.float32)
    Wd = np.asarray(Wd, dtype=np.float32)
    bd = np.asarray(bd, dtype=np.float32)
    Wy = np.asarray(Wy, dtype=np.float32)
    by = np.asarray(by, dtype=np.float32)

    t_steps = U.shape[0]
    nc = _get_nc(t_steps)

    weights = _prep_weights(Wx, bx, Wu, bu, Wd, bd, Wy, by)

    in_maps = []
    for cix in range(N_CORES):
        sl = slice(cix * BL, (cix + 1) * BL)
        xT0 = _to_lhsT_tiles(x[sl].T.copy()).astype(NPBF16)
        in_maps.append({
            "xT0": xT0,
            "UT": _prep_seq(U[:, sl]),
            "DT": _prep_seq(D[:, sl]),
            **weights,
        })

    res = run_bass_kernel_spmd(nc, in_maps, core_ids=list(range(N_CORES)),
                               **run_kwargs)

    X = np.concatenate([res.results[cix]["XO"] for cix in range(N_CORES)],
                       axis=1)
    Y = np.concatenate([res.results[cix]["YO"] for cix in range(N_CORES)],
                       axis=1)
    reg = np.zeros((), dtype=np.float32)
    if run_kwargs:
        kernel.last_results = res
    return X, Y, reg


# revision 8
# speedup vs baseline: 1.2234x; 1.2234x over previous
"""Bass/Trainium2 kernel for nn_BlockSSM (linear block state-space model).

Math (from the reference):
    s_{k+1} = 2*(s_k @ Wx.T + bx + u_k @ Wu.T + bu) + (d_k @ Wd.T + bd)
            = s_k @ A + u_k @ Bu + d_k @ Bd + c
      where A = 2*Wx.T, Bu = 2*Wu.T, Bd = Wd.T, c = 2*bx + 2*bu + bd
    X[k] = s_{k+1}
    Y[k] = s_{k+1} @ Wy.T + by
    reg_error: every stat is multiplied by a 0.0 coefficient (Q_sub's term is
    0.2 * sum([]) == 0), so reg_error == 0.0 exactly.

Strategy: data-parallel over batch (1024 -> 128 per core on 8 cores).
All matmuls run in bf16 with fp32 PSUM accumulation. The state transpose
needed for the next step's matmul (lhsT layout) is done with the DMA xbar
transpose engine, off the compute engines' critical path. The per-step
bias is injected with an all-ones [128,128] lhsT against a bias/128
replicated rhs, so the full state accumulates in PSUM and is evacuated
once (fp32 for the X output via ScalarE, bf16 for the recurrence via
VectorE).
"""

import sys

for _p in ("/opt/trn_rl_repo",):
    if _p not in sys.path:
        sys.path.insert(0, _p)

import numpy as np
import ml_dtypes

import concourse.bacc as bacc
import concourse.mybir as mybir
from concourse.tile import TileContext
from concourse.bass_utils import run_bass_kernel_spmd

BF16 = mybir.dt.bfloat16
F32 = mybir.dt.float32
NPBF16 = ml_dtypes.bfloat16

N_CORES = 8
T, B = 64, 1024
NX, NU, ND, NY = 1024, 512, 256, 256
BL = B // N_CORES  # 128 batch rows per core
KX, KU, KD = NX // 128, NU // 128, ND // 128  # k-tile counts: 8, 4, 2


def build(t_steps: int = T):
    nc = bacc.Bacc("TRN2", target_bir_lowering=False, debug=False,
                   num_devices=N_CORES)

    xT0 = nc.dram_tensor("xT0", [128, NX], BF16, kind="ExternalInput")
    UT = nc.dram_tensor("UT", [t_steps, 128, NU], BF16, kind="ExternalInput")
    DT = nc.dram_tensor("DT", [t_steps, 128, ND], BF16, kind="ExternalInput")
    WA = nc.dram_tensor("WA", [128, KX * NX], BF16, kind="ExternalInput")
    WU = nc.dram_tensor("WU", [128, KU * NX], BF16, kind="ExternalInput")
    WD = nc.dram_tensor("WD", [128, KD * NX], BF16, kind="ExternalInput")
    WY = nc.dram_tensor("WY", [128, KX * NY], BF16, kind="ExternalInput")
    CR = nc.dram_tensor("CR", [128, NX], BF16, kind="ExternalInput")
    BYR = nc.dram_tensor("BYR", [128, NY], BF16, kind="ExternalInput")
    ONE = nc.dram_tensor("ONE", [128, 128], BF16, kind="ExternalInput")
    XO = nc.dram_tensor("XO", [t_steps, 128, NX], F32, kind="ExternalOutput")
    YO = nc.dram_tensor("YO", [t_steps, 128, NY], F32, kind="ExternalOutput")

    with TileContext(nc) as tc:
        with (
            tc.tile_pool(name="wpool", bufs=1) as wpool,
            tc.tile_pool(name="upool", bufs=4) as upool,
            tc.tile_pool(name="dpool", bufs=4) as dpool,
            tc.tile_pool(name="xtpool", bufs=3) as xtpool,
            tc.tile_pool(name="xbfpool", bufs=3) as xbfpool,
            tc.tile_pool(name="xspool", bufs=4) as xspool,
            tc.tile_pool(name="yspool", bufs=4) as yspool,
            tc.tile_pool(name="pxpool", bufs=2, space="PSUM") as pxpool,
            tc.tile_pool(name="pypool", bufs=2, space="PSUM") as pypool,
        ):
            wa = wpool.tile([128, KX * NX], BF16, name="wa")
            nc.gpsimd.dma_start(wa[:], WA[:])
            wu = wpool.tile([128, KU * NX], BF16, name="wu")
            nc.gpsimd.dma_start(wu[:], WU[:])
            wd = wpool.tile([128, KD * NX], BF16, name="wd")
            nc.gpsimd.dma_start(wd[:], WD[:])
            wy = wpool.tile([128, KX * NY], BF16, name="wy")
            nc.gpsimd.dma_start(wy[:], WY[:])
            cr = wpool.tile([128, NX], BF16, name="cr")
            nc.gpsimd.dma_start(cr[:], CR[:])
            byr = wpool.tile([128, NY], BF16, name="byr")
            nc.gpsimd.dma_start(byr[:], BYR[:])
            one = wpool.tile([128, 128], BF16, name="one")
            nc.gpsimd.dma_start(one[:], ONE[:])

            xT_cur = xtpool.tile([128, NX], BF16, name="xT")
            nc.gpsimd.dma_start(xT_cur[:], xT0[:])

            for k in range(t_steps):
                ut = upool.tile([128, NU], BF16, name="ut")
                nc.gpsimd.dma_start(ut[:], UT[k])
                dt = dpool.tile([128, ND], BF16, name="dt")
                nc.gpsimd.dma_start(dt[:], DT[k])

                px_lo = pxpool.tile([128, 512], F32, name="px_lo")
                px_hi = pxpool.tile([128, 512], F32, name="px_hi")
                py = pypool.tile([128, NY], F32, name="py") if k >= 1 else None

                # Y bias via ones-matmul: ones.T @ (by/128 replicated)
                if py is not None:
                    nc.tensor.matmul(py[:], lhsT=one[:], rhs=byr[:],
                                     start=True, stop=False)

                # input projections (independent of the recurrence)
                for kt in range(KU):
                    lh = ut[:, kt * 128:(kt + 1) * 128]
                    nc.tensor.matmul(px_lo[:], lhsT=lh,
                                     rhs=wu[:, kt * NX:kt * NX + 512],
                                     start=(kt == 0), stop=False)
                    nc.tensor.matmul(px_hi[:], lhsT=lh,
                                     rhs=wu[:, kt * NX + 512:(kt + 1) * NX],
                                     start=(kt == 0), stop=False)
                for kt in range(KD):
                    lh = dt[:, kt * 128:(kt + 1) * 128]
                    nc.tensor.matmul(px_lo[:], lhsT=lh,
                                     rhs=wd[:, kt * NX:kt * NX + 512],
                                     start=False, stop=False)
                    nc.tensor.matmul(px_hi[:], lhsT=lh,
                                     rhs=wd[:, kt * NX + 512:(kt + 1) * NX],
                                     start=False, stop=False)

                # recurrence: s_{k+1} += s_k @ A   (critical path)
                for i in range(KX):
                    last = i == KX - 1
                    lh = xT_cur[:, i * 128:(i + 1) * 128]
                    nc.tensor.matmul(px_lo[:], lhsT=lh,
                                     rhs=wa[:, i * NX:i * NX + 512],
                                     start=False, stop=last)
                    nc.tensor.matmul(px_hi[:], lhsT=lh,
                                     rhs=wa[:, i * NX + 512:(i + 1) * NX],
                                     start=False, stop=last)

                # previous step's output head: Y[k-1] = s_k @ Wy.T + by
                # (uses the same lhsT tiles; off the critical path, fills
                # the PE while s_{k+1} is evacuated/transposed)
                if py is not None:
                    for i in range(KX):
                        last = i == KX - 1
                        lh = xT_cur[:, i * 128:(i + 1) * 128]
                        nc.tensor.matmul(py[:], lhsT=lh,
                                         rhs=wy[:, i * NY:(i + 1) * NY],
                                         start=False, stop=last)

                # evacuate state with the c-bias folded in on DVE:
                # bf16 for the recurrence, fp32 for the X output
                xbf = xbfpool.tile([128, NX], BF16, name="xbf")
                nc.vector.tensor_add(out=xbf[:, 0:512], in0=px_lo[:],
                                     in1=cr[:, 0:512])
                nc.vector.tensor_add(out=xbf[:, 512:1024], in0=px_hi[:],
                                     in1=cr[:, 512:1024])
                xstage = xspool.tile([128, NX], F32, name="xstage")
                nc.vector.tensor_add(out=xstage[:, 0:512], in0=px_lo[:],
                                     in1=cr[:, 0:512])
                nc.vector.tensor_add(out=xstage[:, 512:1024], in0=px_hi[:],
                                     in1=cr[:, 512:1024])
                nc.gpsimd.dma_start(XO[k], xstage[:])

                if py is not None:
                    ystage = yspool.tile([128, NY], F32, name="ystage")
                    nc.scalar.copy(out=ystage[:], in_=py[:])
                    nc.gpsimd.dma_start(YO[k - 1], ystage[:])

                # transpose the new state for the next step's lhsT, split
                # across both HWDGE engines (sync + scalar) so the per-tile
                # ~1.28us engine-blocking transposes run two abreast
                xT_next = xtpool.tile([128, NX], BF16, name="xT")
                for i in range(KX):
                    eng = nc.sync if i % 2 == 0 else nc.scalar
                    eng.dma_start_transpose(
                        xT_next[:, i * 128:(i + 1) * 128],
                        xbf[:, i * 128:(i + 1) * 128])
                xT_cur = xT_next

            # epilogue: Y[T-1] = s_T @ Wy.T + by
            py = pypool.tile([128, NY], F32, name="py")
            nc.tensor.matmul(py[:], lhsT=one[:], rhs=byr[:],
                             start=True, stop=False)
            for i in range(KX):
                last = i == KX - 1
                lh = xT_cur[:, i * 128:(i + 1) * 128]
                nc.tensor.matmul(py[:], lhsT=lh,
                                 rhs=wy[:, i * NY:(i + 1) * NY],
                                 start=False, stop=last)
            ystage = yspool.tile([128, NY], F32, name="ystage")
            nc.scalar.copy(out=ystage[:], in_=py[:])
            nc.gpsimd.dma_start(YO[t_steps - 1], ystage[:])

    nc.compile()
    return nc


def _to_lhsT_tiles(mat: np.ndarray) -> np.ndarray:
    """[rows, cols] -> [128, (rows//128)*cols] laid out so that
    out[p, i*cols + j] = mat[i*128 + p, j] (k-tile-major free dim)."""
    rows, cols = mat.shape
    kt = rows // 128
    return (mat.reshape(kt, 128, cols).transpose(1, 0, 2)
            .reshape(128, kt * cols))


def _prep_weights(Wx, bx, Wu, bu, Wd, bd, Wy, by):
    A = (2.0 * Wx.T).astype(np.float32)          # [NX, NX]
    Bu = (2.0 * Wu.T).astype(np.float32)         # [NU, NX]
    Bd = Wd.T.astype(np.float32)                 # [ND, NX]
    Cy = Wy.T.astype(np.float32)                 # [NX, NY]
    c = (2.0 * bx + 2.0 * bu + bd).astype(np.float32)

    wa = _to_lhsT_tiles(A).astype(NPBF16)
    wu = _to_lhsT_tiles(Bu).astype(NPBF16)
    wd = _to_lhsT_tiles(Bd).astype(NPBF16)
    wy = _to_lhsT_tiles(Cy).astype(NPBF16)
    cr = np.broadcast_to(c.astype(NPBF16), (128, NX)).copy()
    byr = np.broadcast_to((by.astype(np.float32) / 128.0).astype(NPBF16),
                          (128, NY)).copy()
    one = np.ones((128, 128), dtype=NPBF16)
    return dict(WA=wa, WU=wu, WD=wd, WY=wy, CR=cr, BYR=byr, ONE=one)


def _prep_seq(M: np.ndarray) -> np.ndarray:
    """[t, BL, F] fp32 -> [t, 128, F] bf16 transposed per step:
    out[t, p, kt*128+m] = M[t, m, kt*128+p]."""
    t, bl, f = M.shape
    kt = f // 128
    out = (M.transpose(0, 2, 1)                 # [t, F, BL]
           .reshape(t, kt, 128, bl)             # [t, kt, p, m]
           .transpose(0, 2, 1, 3)               # [t, p, kt, m]
           .reshape(t, 128, kt * bl))
    return np.ascontiguousarray(out).astype(NPBF16)


_NC_CACHE = {}


def _get_nc(t_steps: int = T):
    if t_steps not in _NC_CACHE:
        _NC_CACHE[t_steps] = build(t_steps)
    return _NC_CACHE[t_steps]


def kernel(x, U, D, Wx, bx, Wu, bu, Wd, bd, Wy, by, **run_kwargs):
    x = np.asarray(x, dtype=np.float32)
    U = np.asarray(U, dtype=np.float32)
    D = np.asarray(D, dtype=np.float32)
    Wx = np.asarray(Wx, dtype=np.float32)
    bx = np.asarray(bx, dtype=np.float32)
    Wu = np.asarray(Wu, dtype=np.float32)
    bu = np.asarray(bu, dtype=np.float32)
    Wd = np.asarray(Wd, dtype=np.float32)
    bd = np.asarray(bd, dtype=np.float32)
    Wy = np.asarray(Wy, dtype=np.float32)
    by = np.asarray(by, dtype=np.float32)

    t_steps = U.shape[0]
    nc = _get_nc(t_steps)

    weights = _prep_weights(Wx, bx, Wu, bu, Wd, bd, Wy, by)

    in_maps = []
    for cix in range(N_CORES):
        sl = slice(cix * BL, (cix + 1) * BL)
        xT0 = _to_lhsT_tiles(x[sl].T.copy()).astype(NPBF16)
        in_maps.append({
            "xT0": xT0,
            "UT": _prep_seq(U[:, sl]),
            "DT": _prep_seq(D[:, sl]),
            **weights,
        })

    res = run_bass_kernel_spmd(nc, in_maps, core_ids=list(range(N_CORES)),
                               **run_kwargs)

    X = np.concatenate([res.results[cix]["XO"] for cix in range(N_CORES)],
                       axis=1)
    Y = np.concatenate([res.results[cix]["YO"] for cix in range(N_CORES)],
                       axis=1)
    reg = np.zeros((), dtype=np.float32)
    if run_kwargs:
        kernel.last_results = res
    return X, Y, reg
